# revision 37
# baseline (speedup 1.0000x reference)
"""Adaptive embedding (4-cluster masked embedding + projection) on 8 trn2 cores.

Sharding: data-parallel over the batch dim - each of the 8 NeuronCores handles
one batch row (2048 tokens); tables replicated.

Design (v5):
- Host does routing only (cluster assignment, stable sort, padded per-tile
  int32 index columns). Device gathers rows with one built-in indirect DMA
  (SWDGE INDIRECT1D) per 128-token tile: the custom dma_gather ucode needs a
  ~12us one-time q7 library load, while INDIRECT1D is built-in and fires
  ~9us into the kernel; per-descriptor cost is ~9-11ns either way, so
  fine-grained per-tile gathers also pipeline the downstream compute.
- Tables are bf16 (emb0 pre-scaled by 32 = sqrt(D_PROJ), exact in bf16);
  gather traffic halves vs fp32.
- Projection weights are fp8e4m3 scaled by 32 (values ~N(0,0.64), well inside
  e4m3 range): validated end-to-end max rel err ~9e-3 against the 2e-2 gate.
- Per tile: PE transpose (bf16) -> PSUM -> DVE/Act evac to SBUF lhsT ->
  bf16 x fp8 matmuls (N=512 x2) -> fp32 PSUM -> evac-cast to fp16 staging.
  Transposes run one tile ahead of the matmuls so the PE never waits on the
  PSUM->SBUF round trip; evacuations alternate Scalar/Vector engines.
- Outputs are written cluster-sorted as a few large fp16 stores (bf16 for
  cluster 0, which needs no projection); host inverse-permutes and upcasts.
"""

import os

import numpy as np
import ml_dtypes

BF16 = ml_dtypes.bfloat16
FP8 = ml_dtypes.float8_e4m3

CUTOFFS = (0, 20000, 40000, 200000, 267735)
D_PROJ = 1024
DES = (1024, 256, 64, 16)
N_CORES = 8
P = 128

_BUILD_CACHE = {}
LAST_RESULT = None


def _build(caps, nwarm, njunk):
    import concourse.bass as bass
    import concourse.bacc as bacc
    import concourse.tile as tile
    from concourse import mybir

    f32 = mybir.dt.float32
    bf16 = mybir.dt.bfloat16
    f16 = mybir.dt.float16
    fp8 = mybir.dt.float8e4
    i32 = mybir.dt.int32

    nts = list(caps)
    ntsum = sum(nts)
    # idx column layout [c0 | c1 | c2 | c3]
    col0 = [0, nts[0], nts[0] + nts[1], nts[0] + nts[1] + nts[2]]

    nc = bacc.Bacc("TRN2", target_bir_lowering=False)
    emb = [
        nc.dram_tensor(f"emb{i}", [CUTOFFS[i + 1] - CUTOFFS[i], DES[i]], bf16,
                       kind="ExternalInput")
        for i in range(4)
    ]
    identd = nc.dram_tensor("ident", [P, P], bf16, kind="ExternalInput")
    proj = [None] + [
        nc.dram_tensor(f"proj{i}", [DES[i], D_PROJ], fp8, kind="ExternalInput")
        for i in (1, 2, 3)
    ]
    idx_all = nc.dram_tensor("idx_all", [P, ntsum], i32, kind="ExternalInput")
    out = [nc.dram_tensor("out0", [nts[0] * P, D_PROJ], bf16, kind="ExternalOutput")] + [
        nc.dram_tensor(f"out{i}", [nts[i] * P, D_PROJ], f16, kind="ExternalOutput")
        for i in (1, 2, 3)
    ]

    with tile.TileContext(nc) as tc:
        with (
            tc.tile_pool(name="const", bufs=1) as cpool,
            tc.tile_pool(name="xt", bufs=6) as xtpool,
            tc.tile_pool(name="tpsum", bufs=2, space="PSUM") as tppool,
            tc.tile_pool(name="wpsum", bufs=1, space="PSUM") as wpool,
            tc.tile_pool(name="mpsum", bufs=2, space="PSUM") as mpool,
        ):
            # identity loaded from DRAM so gpsimd does nothing but gathers
            idxt = cpool.tile([P, ntsum], i32, name="idxt")
            nc.sync.dma_start(out=idxt[:], in_=idx_all[:])
            ident = cpool.tile([P, P], bf16, name="ident")
            nc.scalar.dma_start(out=ident[:], in_=identd[:])

            # weights (fp8) on the scalar engine's HWDGE; c2's first
            w2 = cpool.tile([64, D_PROJ], fp8, name="w2")
            nc.scalar.dma_start(out=w2[:], in_=proj[2][:])
            w1 = [cpool.tile([P, D_PROJ], fp8, name=f"w1_{k}") for k in range(2)]
            for k in range(2):
                nc.scalar.dma_start(out=w1[k][:], in_=proj[1][k * P : (k + 1) * P, :])
            w3 = cpool.tile([16, D_PROJ], fp8, name="w3")
            nc.scalar.dma_start(out=w3[:], in_=proj[3][:])

            # PE warmup: short junk matmuls release the HAM clock gate
            # (1.2 -> 2.4 GHz) before the first real transpose arrives, and
            # inline fillers (below) keep the activity window busy while the
            # PE is paced by the gather stream
            wsrc = cpool.tile([P, P], bf16, name="wsrc")
            nc.vector.memset(wsrc[:], 0.0)
            wps = wpool.tile([P, P], bf16, tag="warm", name="wps")
            for _ in range(nwarm):
                nc.tensor.transpose(out=wps[:], in_=wsrc[:], identity=wsrc[:])

            g = [
                cpool.tile([P, nts[i] * DES[i]], bf16, name=f"g{i}")
                for i in range(4)
            ]

            def gather_tile(i, t):
                de = DES[i]
                nc.gpsimd.indirect_dma_start(
                    out=g[i][:, t * de : (t + 1) * de],
                    out_offset=None,
                    in_=emb[i][:],
                    in_offset=bass.IndirectOffsetOnAxis(
                        ap=idxt[:, col0[i] + t : col0[i] + t + 1], axis=0
                    ),
                )

            # gather order: heavy cluster 2 first; c1 (longest per-tile chain)
            # mid-stream; c3's short chains late; c0 (store-only) last so the
            # kernel tail is minimal
            h2 = nts[2] // 2
            gorder = (
                [(2, t) for t in range(h2)]
                + [(1, t) for t in range(nts[1])]
                + [(2, t) for t in range(h2, nts[2])]
                + [(3, t) for t in range(nts[3])]
                + [(0, t) for t in range(nts[0])]
            )
            for i, t in gorder:
                gather_tile(i, t)

            st = [None] + [
                cpool.tile([P, nts[i] * D_PROJ], f16, name=f"st{i}") for i in (1, 2, 3)
            ]
            pws = {1: w1, 2: [w2], 3: [w3]}
            xev = [0]

            def evac_x(dst, src):
                e = [nc.scalar.copy, nc.vector.tensor_copy][xev[0] % 2]
                xev[0] += 1
                e(out=dst, in_=src)

            oev = [0]

            def evac_out(dst, ps):
                # split each PSUM tile across both engines: balanced by
                # construction and halves the per-tile evac latency
                h = 512
                a, b = (0, h) if oev[0] % 2 else (h, 0)
                oev[0] += 1
                nc.scalar.copy(out=dst[:, a : a + h], in_=ps[:, a : a + h])
                nc.vector.tensor_copy(out=dst[:, b : b + h], in_=ps[:, b : b + h])

            # software-pipelined per-tile compute: transposes run one tile
            # ahead of the matmuls so the PE never stalls on the xt evac
            compute = (
                [(2, t) for t in range(h2)]
                + [(1, t) for t in range(nts[1])]
                + [(2, t) for t in range(h2, nts[2])]
                + [(3, t) for t in range(nts[3])]
            )

            def do_transpose(i, t):
                de = DES[i]
                nk = (de + P - 1) // P
                lhs = []
                for k in range(nk):
                    w = min(P, de - k * P)
                    tp = tppool.tile([w, P], bf16, tag="tp", name=f"tp{i}_{t}_{k}")
                    x = xtpool.tile([w, P], bf16, tag="xt", name=f"xt{i}_{t}_{k}")
                    nc.tensor.transpose(
                        out=tp[:],
                        in_=g[i][:, t * de + k * P : t * de + k * P + w],
                        identity=ident[:],
                    )
                    evac_x(x[:], tp[:])
                    lhs.append(x)
                return lhs

            def do_matmul(i, t, lhs):
                pw = pws[i]
                ps = mpool.tile([P, D_PROJ], f32, tag="ps", name=f"ps{i}_{t}")
                for n in range(2):
                    for k, (lap, pwk) in enumerate(zip(lhs, pw)):
                        nc.tensor.matmul(
                            ps[:, n * 512 : (n + 1) * 512],
                            lap[:],
                            pwk[:, n * 512 : (n + 1) * 512],
                            start=(k == 0),
                            stop=(k == len(lhs) - 1),
                        )
                evac_out(st[i][:, t * D_PROJ : (t + 1) * D_PROJ], ps[:])

            pend = None
            ngp = len(compute) - 4  # tiles still paced by the gather stream
            for j, (i, t) in enumerate(compute):
                lhs = do_transpose(i, t)
                if pend is not None:
                    do_matmul(*pend)
                pend = (i, t, lhs)
                if j < ngp:
                    for _ in range(njunk):
                        nc.tensor.transpose(
                            out=wps[:], in_=wsrc[:], identity=wsrc[:]
                        )
            do_matmul(*pend)

            def store(i, t0, t1):
                dst = out[i][t0 * P : t1 * P, :].rearrange("(t p) i -> p t i", p=P)
                nc.sync.dma_start(out=dst, in_=st[i][:, t0 * D_PROJ : t1 * D_PROJ])

            # stores in tile-completion order, 2-3 tiles each so transfers
            # spread across the gather phase instead of bunching at the end
            def chunks(n, sz):
                return [(a, min(a + sz, n)) for a in range(0, n, sz)]

            for a, b in chunks(h2, 3):
                store(2, a, b)
            store(1, 0, nts[1])
            for a, b in [(a + h2, b + h2) for a, b in chunks(nts[2] - h2, 3)]:
                store(2, a, b)
            for a, b in chunks(nts[3], 3):
                store(3, a, b)
            nc.sync.dma_start(
                out=out[0][:].rearrange("(t p) i -> p t i", p=P),
                in_=g[0][:],
            )

    nc.compile()
    return nc


def kernel(tokens, emb0, emb1, emb2, emb3, proj1, proj2, proj3):
    global LAST_RESULT
    from concourse.bass_utils import run_bass_kernel_spmd

    toks = np.asarray(tokens).astype(np.int64, copy=False)
    nb, ns = toks.shape
    assert nb == N_CORES and ns % P == 0

    scale = np.float32(32.0)  # sqrt(1024): exact power of two, folded in
    embs_b = [
        np.ascontiguousarray((np.asarray(emb0, np.float32) * scale).astype(BF16)),
        np.ascontiguousarray(np.asarray(emb1, np.float32).astype(BF16)),
        np.ascontiguousarray(np.asarray(emb2, np.float32).astype(BF16)),
        np.ascontiguousarray(np.asarray(emb3, np.float32).astype(BF16)),
    ]
    projs_b = {
        i: np.ascontiguousarray((np.asarray(p, np.float32) * scale).astype(FP8))
        for i, p in ((1, proj1), (2, proj2), (3, proj3))
    }

    cuts = np.asarray(CUTOFFS, dtype=np.int64)
    cluster = np.searchsorted(cuts[1:-1], toks, side="right")

    orders, counts, locs = [], [], []
    for c in range(nb):
        cl = cluster[c]
        orders.append(np.argsort(cl, kind="stable"))
        counts.append(np.bincount(cl, minlength=4))
        locs.append((toks[c] - cuts[cl]).astype(np.int32))
    counts = np.stack(counts)

    caps = tuple(
        int(max(1, -(-int(counts[:, i].max()) // P))) for i in range(4)
    )
    nwarm = int(os.environ.get("KERNEL_NWARM", "30"))
    njunk = int(os.environ.get("KERNEL_NJUNK", "12"))
    key = (caps, nwarm, njunk)
    if key not in _BUILD_CACHE:
        _BUILD_CACHE[key] = _build(caps, nwarm, njunk)
    nc = _BUILD_CACHE[key]

    identity = np.ascontiguousarray(np.eye(P, dtype=BF16))
    in_maps = []
    for c in range(nb):
        m = {
            "emb0": embs_b[0], "emb1": embs_b[1],
            "emb2": embs_b[2], "emb3": embs_b[3],
            "proj1": projs_b[1], "proj2": projs_b[2], "proj3": projs_b[3],
            "ident": identity,
        }
        starts = np.concatenate([[0], np.cumsum(counts[c])])
        li = locs[c][orders[c]]
        cols = []
        for i in range(4):
            padded = np.zeros(caps[i] * P, np.int32)
            padded[: counts[c, i]] = li[starts[i] : starts[i + 1]]
            cols.append(padded.reshape(caps[i], P).T)
        m["idx_all"] = np.ascontiguousarray(np.concatenate(cols, axis=1))
        in_maps.append(m)

    res = run_bass_kernel_spmd(nc, in_maps, core_ids=list(range(N_CORES)))
    LAST_RESULT = res

    out = np.empty((nb, ns, D_PROJ), np.float32)
    for c in range(nb):
        segs = [
            np.asarray(res.results[c][f"out{i}"][: counts[c, i]], dtype=np.float32)
            for i in range(4)
        ]
        out[c][orders[c]] = np.concatenate(segs, axis=0)
    return out
